# revision 10
# baseline (speedup 1.0000x reference)
"""Bahdanau additive attention on 8 TRN2 NeuronCores.

Reference computation (B=1, S=65536, H=512, A=256):
    si_proj = si @ Wa.T                       [S, A]
    enc_proj = enc_out @ Ua.T                 [A]
    e = tanh(si_proj + enc_proj) @ va         [S]
    alpha = softmax(e)                        [S]
    context = alpha @ si                      [H]
    returns (context [1,H], alpha [1,S,1])

Distribution: sequence-parallel over S. Each core owns 8192 rows of si,
computes its local unnormalized exp(e) and partial weighted sums, then one
513-float AllReduce combines the softmax normalizer Z and partial context P.
alpha = exp(e - ln Z) and context = P/Z are finalized on-device.

softmax max-subtraction is skipped: |e| <= sum|va| ~ 13, exp(13)*65536 fits
comfortably in f32.
"""

import sys
import numpy as np

for _p in ("/opt/trn_rl_repo",):
    if _p not in sys.path:
        sys.path.insert(0, _p)

import concourse.bass as bass
import concourse.bacc as bacc
import concourse.mybir as mybir
from concourse.tile import TileContext
from concourse.bass_utils import run_bass_kernel_spmd
from concourse.masks import make_identity

F32 = mybir.dt.float32
BF16 = mybir.dt.bfloat16

B, S, H, A = 1, 65536, 512, 256
NCORES = 8
SS = S // NCORES          # rows of si per core = 8192
P = 128                   # partitions
NT = SS // P              # 128-row tiles per core = 64
NB = 8                    # DMA batches of si per core
TPB = NT // NB            # tiles per batch = 8
HC = H // P               # h chunks per tile = 4
EC = (2 * H) // P         # enc chunks = 8

TRACE = False
LAST_EXEC_NS = None
LAST_RESULT = None


def build_nc(ss=SS, nb=NB):
    nt = ss // P
    tpb = nt // nb
    nc = bacc.Bacc(num_devices=NCORES)

    si_ext = nc.declare_dram_parameter("si", [ss, H], F32, isOutput=False)
    wat_ext = nc.declare_dram_parameter("wat", [H, A], F32, isOutput=False)
    uat_ext = nc.declare_dram_parameter("uat", [2 * H, A], F32, isOutput=False)
    enc_ext = nc.declare_dram_parameter("enc", [2 * H], F32, isOutput=False)
    va_ext = nc.declare_dram_parameter("va", [1, A], F32, isOutput=False)
    alpha_ext = nc.declare_dram_parameter("alpha", [P, nt], F32, isOutput=True)
    ctx_ext = nc.declare_dram_parameter("ctx", [1, H], F32, isOutput=True)

    cc_in = nc.dram_tensor("cc_in", [1, H + 1], F32)
    cc_out = nc.dram_tensor("cc_out", [1, H + 1], F32, addr_space="Shared")

    with TileContext(nc) as tc:
        with (
            tc.tile_pool(name="consts", bufs=1) as consts,
            tc.tile_pool(name="sipool", bufs=1) as sipool,
            tc.tile_pool(name="work", bufs=3) as work,
            tc.tile_pool(name="pproj", bufs=2, space="PSUM") as pprojp,
            tc.tile_pool(name="pacc", bufs=1, space="PSUM") as paccp,
            tc.tile_pool(name="pmisc", bufs=1, space="PSUM") as pmiscp,
        ):
            # ---- constants / prologue -------------------------------------
            ones_row = consts.tile([1, P], BF16)
            nc.vector.memset(ones_row[:], 1.0)
            ones_col = consts.tile([P, 1], F32)
            nc.vector.memset(ones_col[:], 1.0)

            wat_sb = consts.tile([P, HC * A], BF16)
            nc.gpsimd.dma_start(
                out=wat_sb.rearrange("p (c a) -> p c a", c=HC),
                in_=wat_ext.rearrange("(c p) a -> p c a", p=P),
            )
            uat_sb = consts.tile([P, EC * A], BF16)
            nc.gpsimd.dma_start(
                out=uat_sb.rearrange("p (c a) -> p c a", c=EC),
                in_=uat_ext.rearrange("(c p) a -> p c a", p=P),
            )
            enc_sb = consts.tile([P, EC], BF16)
            nc.gpsimd.dma_start(
                out=enc_sb[:], in_=enc_ext.rearrange("(c p) -> p c", p=P)
            )
            va_sb = consts.tile([1, A], BF16)
            nc.gpsimd.dma_start(out=va_sb[:], in_=va_ext[:])

            # enc_proj = enc_out @ Ua.T   -> [1, A]
            pencp = pmiscp.tile([1, A], F32, tag="misc")
            for c in range(EC):
                nc.tensor.matmul(
                    pencp[:],
                    enc_sb[:, c : c + 1],
                    uat_sb[:, c * A : (c + 1) * A],
                    start=(c == 0),
                    stop=(c == EC - 1),
                )
            encp_row = consts.tile([1, A], BF16)
            nc.vector.tensor_copy(encp_row[:], pencp[:])

            # va broadcast [P, 4A], pre-scaled by 2 (tanh-via-sigmoid)
            pbc2 = pmiscp.tile([P, A], F32, tag="misc")
            nc.tensor.matmul(pbc2[:], ones_row[:], va_sb[:])
            va_bc4 = consts.tile([P, 4 * A], BF16)
            for q in range(4):
                nc.vector.tensor_scalar_mul(
                    va_bc4[:, q * A : (q + 1) * A], pbc2[:], 2.0
                )

            # ---- stream si in (cast f32->bf16) and xbar-transpose ---------
            si_b, siT_b = [], []
            for b in range(nb):
                t_sb = sipool.tile([P, tpb * H], BF16, name=f"si_b{b}")
                nc.gpsimd.dma_start(
                    out=t_sb.rearrange("p (n h) -> p n h", n=tpb),
                    in_=si_ext[b * tpb * P : (b + 1) * tpb * P, :].rearrange(
                        "(n p) h -> p n h", p=P
                    ),
                )
                si_b.append(t_sb)
                tT_sb = sipool.tile([P, tpb * H], BF16, name=f"siT_b{b}")
                nc.sync.dma_start(
                    out=tT_sb.rearrange("p (n f) -> p n f", n=tpb * HC),
                    in_=t_sb[:],
                    transpose=True,
                )
                siT_b.append(tT_sb)

            e_all = consts.tile([P, nt], F32)
            expb = consts.tile([P, nt], BF16)
            zcol = consts.tile([P, 1], F32)
            pP = paccp.tile([1, H], F32)

            # ---- main loop: scores ----------------------------------------
            th4 = None
            pproj2 = None
            for t in range(nt):
                b, k = t // tpb, t % tpb
                if t % 2 == 0:
                    pproj2 = pprojp.tile([P, 2 * A], F32)
                half = (t % 2) * A
                dst = pproj2[:, half : half + A]
                for hc in range(HC):
                    nc.tensor.matmul(
                        dst,
                        siT_b[b][:, (k * HC + hc) * P : (k * HC + hc + 1) * P],
                        wat_sb[:, hc * A : (hc + 1) * A],
                        start=(hc == 0),
                        stop=False,
                    )
                # += 1 (x) enc_proj  (rank-1 accumulate)
                nc.tensor.matmul(
                    dst, ones_row[:], encp_row[:], start=False, stop=True
                )
                if t % 2 == 1:
                    if t % 4 == 1:
                        th4 = work.tile([P, 4 * A], BF16)
                    q = (t // 2) % 2
                    nc.scalar.activation(
                        th4[:, q * 2 * A : (q + 1) * 2 * A],
                        pproj2[:],
                        mybir.ActivationFunctionType.Sigmoid,
                        scale=2.0,
                    )
                if t % 4 == 3:
                    prod4 = work.tile([P, 4 * A], BF16)
                    nc.vector.tensor_mul(prod4[:], th4[:], va_bc4[:])
                    nc.vector.reduce_sum(
                        e_all[:, t - 3 : t + 1],
                        prod4.rearrange("p (n a) -> p n a", n=4),
                        axis=mybir.AxisListType.X,
                    )

            # ---- tail: exp, weighted sum, allreduce, outputs --------------
            nc.scalar.activation(
                expb[:],
                e_all[:],
                mybir.ActivationFunctionType.Exp,
                accum_out=zcol[:],
            )
            for t in range(nt):
                b, k = t // tpb, t % tpb
                nc.tensor.matmul(
                    pP[:],
                    expb[:, t : t + 1],
                    si_b[b][:, k * H : (k + 1) * H],
                    start=(t == 0),
                    stop=(t == nt - 1),
                    skip_group_check=True,
                )
            pZ = pmiscp.tile([1, 1], F32, tag="misc")
            nc.tensor.matmul(pZ[:], zcol[:], ones_col[:])

            cc_sb = consts.tile([1, H + 1], F32)
            nc.vector.tensor_copy(cc_sb[:, :H], pP[:])
            nc.vector.tensor_copy(cc_sb[:, H : H + 1], pZ[:])
            nc.sync.dma_start(out=cc_in[:], in_=cc_sb[:])
            nc.gpsimd.collective_compute(
                "AllReduce",
                mybir.AluOpType.add,
                replica_groups=[list(range(NCORES))],
                ins=[cc_in[:]],
                outs=[cc_out[:]],
            )
            ccr_sb = consts.tile([1, H + 1], F32)
            nc.sync.dma_start(out=ccr_sb[:], in_=cc_out[:])

            invZ = consts.tile([1, 1], F32)
            nc.vector.reciprocal(invZ[:], ccr_sb[:, H : H + 1])
            ctx_sb = consts.tile([1, H], F32)
            nc.vector.tensor_scalar_mul(ctx_sb[:], ccr_sb[:, :H], invZ[:])
            nc.sync.dma_start(out=ctx_ext[:], in_=ctx_sb[:])

            lnZ = consts.tile([1, 1], F32)
            nc.scalar.activation(
                lnZ[:], ccr_sb[:, H : H + 1], mybir.ActivationFunctionType.Ln
            )
            nlnZ = consts.tile([1, 1], F32)
            nc.vector.tensor_scalar_mul(nlnZ[:], lnZ[:], -1.0)
            onesf_row = consts.tile([1, P], F32)
            nc.vector.memset(onesf_row[:], 1.0)
            pbias = pmiscp.tile([P, 1], F32, tag="misc")
            nc.tensor.matmul(pbias[:], onesf_row[:], nlnZ[:])
            bias_sb = consts.tile([P, 1], F32)
            nc.vector.tensor_copy(bias_sb[:], pbias[:])
            alpha_sb = consts.tile([P, nt], F32)
            nc.scalar.activation(
                alpha_sb[:],
                e_all[:],
                mybir.ActivationFunctionType.Exp,
                bias=bias_sb[:],
            )
            nc.sync.dma_start(out=alpha_ext[:], in_=alpha_sb[:])

    nc.compile()
    return nc


_NC_CACHE = None


def kernel(enc_out, si, Wa, Ua, va):
    global LAST_EXEC_NS, LAST_RESULT, _NC_CACHE

    enc_out = np.ascontiguousarray(np.asarray(enc_out, dtype=np.float32))
    si = np.ascontiguousarray(np.asarray(si, dtype=np.float32))
    Wa = np.asarray(Wa, dtype=np.float32)
    Ua = np.asarray(Ua, dtype=np.float32)
    va = np.asarray(va, dtype=np.float32)

    si2 = si.reshape(S, H)
    wat = np.ascontiguousarray(Wa.T)          # [H, A]
    uat = np.ascontiguousarray(Ua.T)          # [2H, A]
    enc = np.ascontiguousarray(enc_out.reshape(2 * H))
    va_row = np.ascontiguousarray(va.reshape(1, A))

    if _NC_CACHE is None:
        _NC_CACHE = build_nc()
    nc = _NC_CACHE

    in_maps = []
    for i in range(NCORES):
        in_maps.append(
            {
                "si": np.ascontiguousarray(si2[i * SS : (i + 1) * SS, :]),
                "wat": wat,
                "uat": uat,
                "enc": enc,
                "va": va_row,
            }
        )

    res = run_bass_kernel_spmd(nc, in_maps, list(range(NCORES)), trace=TRACE)
    LAST_EXEC_NS = res.exec_time_ns
    LAST_RESULT = res

    alpha_full = np.empty((S,), dtype=np.float32)
    for i in range(NCORES):
        a = np.asarray(res.results[i]["alpha"])          # [P, NT]
        alpha_full[i * SS : (i + 1) * SS] = a.T.reshape(SS)
    context = np.asarray(res.results[0]["ctx"]).reshape(1, H)
    return context, alpha_full.reshape(1, S, 1)


# revision 12
# speedup vs baseline: 1.4675x; 1.4675x over previous
"""Bahdanau additive attention on 8 TRN2 NeuronCores.

Reference computation (B=1, S=65536, H=512, A=256):
    si_proj = si @ Wa.T                       [S, A]
    enc_proj = enc_out @ Ua.T                 [A]
    e = tanh(si_proj + enc_proj) @ va         [S]
    alpha = softmax(e)                        [S]
    context = alpha @ si                      [H]
    returns (context [1,H], alpha [1,S,1])

Distribution: sequence-parallel over S. Each core owns 8192 rows of si,
computes its local unnormalized exp(e) and partial weighted sums, then one
513-float AllReduce combines the softmax normalizer Z and partial context P.
alpha = exp(e - ln Z) and context = P/Z are finalized on-device.

softmax max-subtraction is skipped: |e| <= sum|va| ~ 13, exp(13)*65536 fits
comfortably in f32.
"""

import sys
import numpy as np

for _p in ("/opt/trn_rl_repo",):
    if _p not in sys.path:
        sys.path.insert(0, _p)

import concourse.bass as bass
import concourse.bacc as bacc
import concourse.mybir as mybir
from concourse.tile import TileContext
from concourse.bass_utils import run_bass_kernel_spmd
from concourse.masks import make_identity

F32 = mybir.dt.float32
BF16 = mybir.dt.bfloat16

B, S, H, A = 1, 65536, 512, 256
NCORES = 8
SS = S // NCORES          # rows of si per core = 8192
P = 128                   # partitions
NT = SS // P              # 128-row tiles per core = 64
NB = 8                    # DMA batches of si per core
TPB = NT // NB            # tiles per batch = 8
HC = H // P               # h chunks per tile = 4
EC = (2 * H) // P         # enc chunks = 8

TRACE = False
LAST_EXEC_NS = None
LAST_RESULT = None


def build_nc(ss=SS, nb=NB):
    nt = ss // P
    tpb = nt // nb
    nc = bacc.Bacc(num_devices=NCORES)

    si_ext = nc.declare_dram_parameter("si", [ss, H], F32, isOutput=False)
    wat_ext = nc.declare_dram_parameter("wat", [H, A], F32, isOutput=False)
    uat_ext = nc.declare_dram_parameter("uat", [2 * H, A], F32, isOutput=False)
    enc_ext = nc.declare_dram_parameter("enc", [2 * H], F32, isOutput=False)
    va_ext = nc.declare_dram_parameter("va", [1, A], F32, isOutput=False)
    alpha_ext = nc.declare_dram_parameter("alpha", [P, nt], F32, isOutput=True)
    ctx_ext = nc.declare_dram_parameter("ctx", [1, H], F32, isOutput=True)

    cc_in = nc.dram_tensor("cc_in", [1, H + 1], F32)
    cc_out = nc.dram_tensor("cc_out", [1, H + 1], F32, addr_space="Shared")

    with TileContext(nc) as tc:
        with (
            tc.tile_pool(name="consts", bufs=1) as consts,
            tc.tile_pool(name="sipool", bufs=1) as sipool,
            tc.tile_pool(name="work", bufs=3) as work,
            tc.tile_pool(name="pt", bufs=2, space="PSUM") as ptp,
            tc.tile_pool(name="pproj", bufs=2, space="PSUM") as pprojp,
            tc.tile_pool(name="pacc", bufs=1, space="PSUM") as paccp,
            tc.tile_pool(name="pmisc", bufs=1, space="PSUM") as pmiscp,
        ):
            # ---- constants / prologue -------------------------------------
            ones_row = consts.tile([1, P], BF16)
            nc.vector.memset(ones_row[:], 1.0)
            ones_col = consts.tile([P, 1], F32)
            nc.vector.memset(ones_col[:], 1.0)

            # warm the PE HAM clock gate with a dense dummy matmul burst
            warm_sb = consts.tile([P, 2 * A], BF16)
            nc.vector.memset(warm_sb[:], 0.0)
            pwarm = pmiscp.tile([P, 2 * A], F32, tag="warm")
            for w in range(24):
                nc.tensor.matmul(
                    pwarm[:],
                    warm_sb[:, :P],
                    warm_sb[:],
                    start=True,
                    stop=True,
                    skip_group_check=True,
                )

            wat_sb = consts.tile([P, HC * A], BF16)
            nc.gpsimd.dma_start(
                out=wat_sb.rearrange("p (c a) -> p c a", c=HC),
                in_=wat_ext.rearrange("(c p) a -> p c a", p=P),
            )
            uat_sb = consts.tile([P, EC * A], BF16)
            nc.gpsimd.dma_start(
                out=uat_sb.rearrange("p (c a) -> p c a", c=EC),
                in_=uat_ext.rearrange("(c p) a -> p c a", p=P),
            )
            enc_sb = consts.tile([P, EC], BF16)
            nc.gpsimd.dma_start(
                out=enc_sb[:], in_=enc_ext.rearrange("(c p) -> p c", p=P)
            )
            va_sb = consts.tile([1, A], BF16)
            nc.gpsimd.dma_start(out=va_sb[:], in_=va_ext[:])

            # enc_proj = enc_out @ Ua.T   -> [1, A]
            pencp = pmiscp.tile([1, A], F32, tag="misc")
            for c in range(EC):
                nc.tensor.matmul(
                    pencp[:],
                    enc_sb[:, c : c + 1],
                    uat_sb[:, c * A : (c + 1) * A],
                    start=(c == 0),
                    stop=(c == EC - 1),
                )
            encp_row2 = consts.tile([1, 2 * A], BF16)
            nc.vector.tensor_copy(encp_row2[:, :A], pencp[:])
            nc.vector.tensor_copy(encp_row2[:, A:], pencp[:])

            # va broadcast [P, 4A], pre-scaled by 2 (tanh-via-sigmoid)
            pbc2 = pmiscp.tile([P, A], F32, tag="misc")
            nc.tensor.matmul(pbc2[:], ones_row[:], va_sb[:])
            va_bc4 = consts.tile([P, 4 * A], BF16)
            for q in range(4):
                nc.vector.tensor_scalar_mul(
                    va_bc4[:, q * A : (q + 1) * A], pbc2[:], 2.0
                )

            # ---- stream si in (cast f32->bf16) and xbar-transpose ---------
            si_b = []
            for b in range(nb):
                t_sb = sipool.tile([P, tpb * H], BF16, name=f"si_b{b}")
                nc.gpsimd.dma_start(
                    out=t_sb.rearrange("p (n h) -> p n h", n=tpb),
                    in_=si_ext[b * tpb * P : (b + 1) * tpb * P, :].rearrange(
                        "(n p) h -> p n h", p=P
                    ),
                )
                si_b.append(t_sb)

            e_all = consts.tile([P, nt], F32)
            expb = consts.tile([P, nt], BF16)
            zcol = consts.tile([P, 1], F32)
            pP = paccp.tile([1, H], F32)

            # ---- main loop: scores ----------------------------------------
            ident = consts.tile([P, P], BF16)
            make_identity(nc, ident)
            th4 = None
            pproj2 = None
            pT2 = None
            siT2 = None
            for t in range(nt):
                b, k = t // tpb, t % tpb
                si_slice = si_b[b][:, k * H : (k + 1) * H]
                if t % 2 == 0:
                    pT2 = ptp.tile([P, 2 * H], BF16)
                    siT2 = work.tile([P, 2 * H], BF16, tag="siT2")
                    pproj2 = pprojp.tile([P, 2 * A], F32)
                hh = (t % 2) * H
                for c in range(HC):
                    nc.tensor.transpose(
                        pT2[:, hh + c * P : hh + (c + 1) * P],
                        si_slice[:, c * P : (c + 1) * P],
                        ident[:],
                    )
                if t % 2 == 1:
                    nc.scalar.copy(siT2[:], pT2[:])
                    # group opener: pproj2 = 1 (x) enc_proj (both tiles)
                    nc.tensor.matmul(
                        pproj2[:],
                        ones_row[:],
                        encp_row2[:],
                        start=True,
                        stop=False,
                        skip_group_check=True,
                    )
                    for tt in (t - 1, t):
                        half = (tt % 2) * A
                        hh2 = (tt % 2) * H
                        for hc in range(HC):
                            nc.tensor.matmul(
                                pproj2[:, half : half + A],
                                siT2[:, hh2 + hc * P : hh2 + (hc + 1) * P],
                                wat_sb[:, hc * A : (hc + 1) * A],
                                start=False,
                                stop=(tt == t and hc == HC - 1),
                                skip_group_check=True,
                            )
                    if t % 4 == 1:
                        th4 = work.tile([P, 4 * A], BF16)
                    q = (t // 2) % 2
                    nc.scalar.activation(
                        th4[:, q * 2 * A : (q + 1) * 2 * A],
                        pproj2[:],
                        mybir.ActivationFunctionType.Sigmoid,
                        scale=2.0,
                    )
                if t % 4 == 3:
                    prod4 = work.tile([P, 4 * A], BF16)
                    nc.vector.tensor_mul(prod4[:], th4[:], va_bc4[:])
                    nc.vector.reduce_sum(
                        e_all[:, t - 3 : t + 1],
                        prod4.rearrange("p (n a) -> p n a", n=4),
                        axis=mybir.AxisListType.X,
                    )

            # ---- tail: exp, weighted sum, allreduce, outputs --------------
            nc.scalar.activation(
                expb[:],
                e_all[:],
                mybir.ActivationFunctionType.Exp,
                accum_out=zcol[:],
            )
            for t in range(nt):
                b, k = t // tpb, t % tpb
                nc.tensor.matmul(
                    pP[:],
                    expb[:, t : t + 1],
                    si_b[b][:, k * H : (k + 1) * H],
                    start=(t == 0),
                    stop=(t == nt - 1),
                    skip_group_check=True,
                )
            pZ = pmiscp.tile([1, 1], F32, tag="misc")
            nc.tensor.matmul(pZ[:], zcol[:], ones_col[:])

            cc_sb = consts.tile([1, H + 1], F32)
            nc.vector.tensor_copy(cc_sb[:, :H], pP[:])
            nc.vector.tensor_copy(cc_sb[:, H : H + 1], pZ[:])
            nc.sync.dma_start(out=cc_in[:], in_=cc_sb[:])
            nc.gpsimd.collective_compute(
                "AllReduce",
                mybir.AluOpType.add,
                replica_groups=[list(range(NCORES))],
                ins=[cc_in[:]],
                outs=[cc_out[:]],
            )
            ccr_sb = consts.tile([1, H + 1], F32)
            nc.sync.dma_start(out=ccr_sb[:], in_=cc_out[:])

            invZ = consts.tile([1, 1], F32)
            nc.vector.reciprocal(invZ[:], ccr_sb[:, H : H + 1])
            ctx_sb = consts.tile([1, H], F32)
            nc.vector.tensor_scalar_mul(ctx_sb[:], ccr_sb[:, :H], invZ[:])
            nc.sync.dma_start(out=ctx_ext[:], in_=ctx_sb[:])

            lnZ = consts.tile([1, 1], F32)
            nc.scalar.activation(
                lnZ[:], ccr_sb[:, H : H + 1], mybir.ActivationFunctionType.Ln
            )
            nlnZ = consts.tile([1, 1], F32)
            nc.vector.tensor_scalar_mul(nlnZ[:], lnZ[:], -1.0)
            onesf_row = consts.tile([1, P], F32)
            nc.vector.memset(onesf_row[:], 1.0)
            pbias = pmiscp.tile([P, 1], F32, tag="misc")
            nc.tensor.matmul(pbias[:], onesf_row[:], nlnZ[:])
            bias_sb = consts.tile([P, 1], F32)
            nc.vector.tensor_copy(bias_sb[:], pbias[:])
            alpha_sb = consts.tile([P, nt], F32)
            nc.scalar.activation(
                alpha_sb[:],
                e_all[:],
                mybir.ActivationFunctionType.Exp,
                bias=bias_sb[:],
            )
            nc.sync.dma_start(out=alpha_ext[:], in_=alpha_sb[:])

    nc.compile()
    return nc


_NC_CACHE = None


def kernel(enc_out, si, Wa, Ua, va):
    global LAST_EXEC_NS, LAST_RESULT, _NC_CACHE

    enc_out = np.ascontiguousarray(np.asarray(enc_out, dtype=np.float32))
    si = np.ascontiguousarray(np.asarray(si, dtype=np.float32))
    Wa = np.asarray(Wa, dtype=np.float32)
    Ua = np.asarray(Ua, dtype=np.float32)
    va = np.asarray(va, dtype=np.float32)

    si2 = si.reshape(S, H)
    wat = np.ascontiguousarray(Wa.T)          # [H, A]
    uat = np.ascontiguousarray(Ua.T)          # [2H, A]
    enc = np.ascontiguousarray(enc_out.reshape(2 * H))
    va_row = np.ascontiguousarray(va.reshape(1, A))

    if _NC_CACHE is None:
        _NC_CACHE = build_nc()
    nc = _NC_CACHE

    in_maps = []
    for i in range(NCORES):
        in_maps.append(
            {
                "si": np.ascontiguousarray(si2[i * SS : (i + 1) * SS, :]),
                "wat": wat,
                "uat": uat,
                "enc": enc,
                "va": va_row,
            }
        )

    res = run_bass_kernel_spmd(nc, in_maps, list(range(NCORES)), trace=TRACE)
    LAST_EXEC_NS = res.exec_time_ns
    LAST_RESULT = res

    alpha_full = np.empty((S,), dtype=np.float32)
    for i in range(NCORES):
        a = np.asarray(res.results[i]["alpha"])          # [P, NT]
        alpha_full[i * SS : (i + 1) * SS] = a.T.reshape(SS)
    context = np.asarray(res.results[0]["ctx"]).reshape(1, H)
    return context, alpha_full.reshape(1, S, 1)
